# revision 7
# baseline (speedup 1.0000x reference)
"""DLRM forward (embedding_lookup) on 8 Trainium2 NeuronCores.

Strategy: batch-parallel SPMD. Each core owns B/8 = 512 samples and a full
replica of the stacked embedding tables. Per core:
  - bottom MLP (13->512->256->128) in [features, samples] orientation
  - embedding gather via wide indirect DMAs (1 MB each: 4 tables x 1
    sample-chunk x 4 indices x 128 samples), pooling-by-4 via one strided
    DVE tensor_reduce per (table, chunk)
  - PE transpose pooled [samples, D] -> [D, samples] into a per-chunk
    concatenated T matrix [128, 27*128]
  - pairwise interaction as per-sample 27x27 matmuls (strided lhsT AP)
  - lower-triangle extraction via 27 strided PSUM->SBUF copies per half-chunk
  - top MLP (479->1024->1024->512->256->1) with the tril selection absorbed
    into a host-side permutation of the first layer's weight columns
No cross-core collectives are needed.
"""

import os
import sys

import numpy as np

for _p in ("/opt/trn_rl_repo", "/root/.axon_site/_ro/trn_rl_repo"):
    if os.path.isdir(_p) and _p not in sys.path:
        sys.path.append(_p)

import concourse.bass as bass
import concourse.mybir as mybir
import concourse.tile as tile
from concourse.bass_utils import run_bass_kernel_spmd
from concourse.masks import make_identity

T = 26          # embedding tables
V = 100000      # vocab per table
D = 128         # embedding dim
B = 4096        # global batch
NCORES = 8
S = B // NCORES         # samples per core
CHUNKS = 4              # 128-sample chunks per core
P = 128
NV = T + 1              # vectors per sample (x + 26 tables)
NZ = (NV * (NV - 1)) // 2   # 351 strictly-lower interaction features
NF = D + NZ             # 479 top-MLP input features (reference)
NZTILES = 7             # z K-tiles: 4 column-groups per tile at 32-aligned slots
NFP = D + NZTILES * P   # 1024: padded top-MLP K dim on device
FP = mybir.dt.float32
I32 = mybir.dt.int32

# gather DMA groups per chunk: tables packed 4 per DMA (16 idx/partition = 1 MB)
GROUPS = [(0, 4), (4, 4), (8, 4), (12, 4), (16, 4), (20, 4), (24, 2)]

BOT_DIMS = [13, 512, 256, 128]
TOP_DIMS = [NFP, 1024, 1024, 512, 256, 1]


def _tj(j):
    # feature offset of Z column-group j in our j-major tril ordering
    return 26 * j - (j * (j - 1)) // 2


def _split_multi_waits(nc, max_waits=1):
    """Walrus codegen here only accepts one sync wait per instruction.
    Hoist extra waits onto same-engine NoOps placed just before."""
    for f in nc.m.functions:
        for blk in f.blocks:
            new_insts = []
            for inst in blk.instructions:
                si = inst.sync_info
                if si is not None and si.on_wait and len(si.on_wait) > max_waits:
                    waits = list(si.on_wait)
                    for i, w in enumerate(waits[:-max_waits]):
                        new_insts.append(
                            mybir.InstNoOp(
                                name=f"{inst.name}-ws{i}",
                                engine=inst.engine,
                                sync_info=mybir.SyncInfo(on_wait=[w], on_update=[]),
                                bass_nofuse=True,
                            )
                        )
                    si.on_wait = waits[-max_waits:]
                new_insts.append(inst)
            blk.instructions = new_insts


def _build_nc():
    nc = bass.Bass()
    tbl = nc.declare_dram_parameter("tbl", [T * V, D], FP, isOutput=False)
    idx_d = nc.declare_dram_parameter("idx", [P, CHUNKS * T * 4], I32, isOutput=False)
    dxt_d = nc.declare_dram_parameter("dxt", [BOT_DIMS[0], S], FP, isOutput=False)
    # bottom weights: per layer one tile [128, nk*M]; biases [128, n_mtiles]
    bw_d, bb_d = [], []
    for li in range(3):
        din, dout = BOT_DIMS[li], BOT_DIMS[li + 1]
        nk = max(1, din // P)
        kp = min(din, P)
        bw_d.append(nc.declare_dram_parameter(f"bw{li}", [kp, nk * dout], FP, isOutput=False))
        bb_d.append(nc.declare_dram_parameter(f"bb{li}", [P, max(1, dout // P)], FP, isOutput=False))
    tw_d, tb_d = [], []
    for li in range(5):
        din, dout = TOP_DIMS[li], TOP_DIMS[li + 1]
        nk = (din + P - 1) // P
        tw_d.append(nc.declare_dram_parameter(f"tw{li}", [P, nk * dout], FP, isOutput=False))
        tb_d.append(nc.declare_dram_parameter(f"tb{li}", [P, max(1, dout // P)], FP, isOutput=False))
    y_d = nc.declare_dram_parameter("y", [1, S], FP, isOutput=True)

    with tile.TileContext(nc) as tc:
        with (
            tc.tile_pool(name="wp", bufs=1) as wp,
            tc.tile_pool(name="gp", bufs=3) as gp,
            tc.tile_pool(name="pp", bufs=3) as pp,
            tc.tile_pool(name="ttp", bufs=2) as ttp,
            tc.tile_pool(name="hp", bufs=8) as hp,
            tc.tile_pool(name="ktp", bufs=1) as ktp,
            tc.tile_pool(name="zp", bufs=1, space="PSUM") as zpp,
            tc.tile_pool(name="trp", bufs=2, space="PSUM") as trpp,
            tc.tile_pool(name="mp", bufs=2, space="PSUM") as mpp,
        ):
            # ---- resident loads -------------------------------------------
            idx_t = wp.tile([P, CHUNKS * T * 4], I32, tag="idx")
            nc.sync.dma_start(out=idx_t[:], in_=idx_d[:])
            dxt_t = wp.tile([BOT_DIMS[0], S], FP, tag="dxt")
            nc.sync.dma_start(out=dxt_t[:], in_=dxt_d[:])
            ident = wp.tile([P, P], FP, tag="ident")
            make_identity(nc, ident[:])
            bw_t, bb_t, tw_t, tb_t = [], [], [], []
            for li in range(3):
                w = wp.tile(list(bw_d[li].shape), FP, tag=f"bw{li}", name=f"bwt{li}")
                nc.sync.dma_start(out=w[:], in_=bw_d[li][:])
                bw_t.append(w)
                b = wp.tile(list(bb_d[li].shape), FP, tag=f"bb{li}", name=f"bbt{li}")
                nc.sync.dma_start(out=b[:], in_=bb_d[li][:])
                bb_t.append(b)
            for li in range(5):
                w = wp.tile(list(tw_d[li].shape), FP, tag=f"tw{li}", name=f"twt{li}")
                nc.sync.dma_start(out=w[:], in_=tw_d[li][:])
                tw_t.append(w)
                b = wp.tile(list(tb_d[li].shape), FP, tag=f"tb{li}", name=f"tbt{li}")
                nc.sync.dma_start(out=b[:], in_=tb_d[li][:])
                tb_t.append(b)

            # ---- bottom MLP: dxt [13, S] -> xall [128, S] ------------------
            def mlp_layer(h_in_tiles, kdim, w_tile, b_tile, dout, func, out_tiles, kp_first=None):
                nk = len(h_in_tiles)
                nm = (dout + P - 1) // P
                for m in range(nm):
                    mw = min(P, dout - m * P)
                    ps = mpp.tile([P, S], FP, tag="mlp")
                    for k in range(nk):
                        kp = kp_first if (kp_first is not None and nk == 1) else min(P, kdim - k * P)
                        nc.tensor.matmul(
                            out=ps[0:mw, :],
                            lhsT=w_tile[0:kp, k * dout + m * P : k * dout + m * P + mw],
                            rhs=h_in_tiles[k][0:kp, :],
                            start=(k == 0),
                            stop=(k == nk - 1),
                        )
                    nc.scalar.activation(
                        out=out_tiles[m][0:mw, :],
                        in_=ps[0:mw, :],
                        func=func,
                        bias=b_tile[0:mw, m : m + 1],
                    )

            relu = mybir.ActivationFunctionType.Relu
            h1 = [hp.tile([P, S], FP, tag="hA", name=f"h1_{i}") for i in range(4)]
            mlp_layer([dxt_t], 13, bw_t[0], bb_t[0], 512, relu, h1, kp_first=13)
            h2 = [hp.tile([P, S], FP, tag="hB", name=f"h2_{i}") for i in range(2)]
            mlp_layer(h1, 512, bw_t[1], bb_t[1], 256, relu, h2)
            xall = ktp.tile([P, S], FP, tag="kt0")
            mlp_layer(h2, 256, bw_t[2], bb_t[2], 128, relu, [xall])

            # ---- R feature K-tiles (zeroed: unwritten slot rows must be 0) -
            kt = [ktp.tile([P, S], FP, tag=f"kt{n}", name=f"kt_{n}")
                  for n in range(1, NZTILES + 1)]
            for n in range(NZTILES):
                nc.gpsimd.memset(kt[n][:], 0.0)

            # ---- per chunk: gather -> pool -> transpose -> interact -> extract
            for cc in range(CHUNKS):
                tt = ttp.tile([P, NV * P], FP, tag="tt")
                # x block
                nc.vector.tensor_copy(out=tt[:, 0:P], in_=xall[:, cc * P : (cc + 1) * P])
                for (t0, nt) in GROUPS:
                    g = gp.tile([P, 4 * nt * P], FP, tag="g")
                    c0 = cc * (T * 4) + t0 * 4
                    nc.gpsimd.indirect_dma_start(
                        out=g[:],
                        out_offset=None,
                        in_=tbl[:],
                        in_offset=bass.IndirectOffsetOnAxis(
                            ap=idx_t[:, c0 : c0 + 4 * nt], axis=0
                        ),
                    )
                    for gi in range(nt):
                        t = t0 + gi
                        pt = pp.tile([P, P], FP, tag="p")
                        src = g[:, gi * 512 : (gi + 1) * 512].rearrange(
                            "p (j d) -> p d j", j=4
                        )
                        nc.vector.tensor_reduce(
                            out=pt[:], in_=src, op=mybir.AluOpType.add,
                            axis=mybir.AxisListType.X,
                        )
                        tr = trpp.tile([P, P], FP, tag="tr")
                        nc.tensor.transpose(out=tr[:], in_=pt[:], identity=ident[:])
                        nc.vector.tensor_copy(
                            out=tt[:, (t + 1) * P : (t + 2) * P], in_=tr[:]
                        )
                # interaction + extraction in two 64-sample halves
                for h in range(2):
                    zp = zpp.tile([P, 2048], FP, tag="z")
                    for sl in range(64):
                        s = h * 64 + sl
                        lhsT = tt[:, s :: P]
                        nc.tensor.matmul(
                            out=zp[0:NV, 32 * sl : 32 * sl + NV],
                            lhsT=lhsT, rhs=lhsT, start=True, stop=True,
                        )
                    cols = slice(cc * P + h * 64, cc * P + h * 64 + 64)
                    # column-group j of Z -> 32-aligned slot (engines require
                    # 32-aligned partition bases); upper-triangle/diagonal
                    # rows are killed by zeroed W1 columns host-side
                    for j in range(NV - 1):
                        nc.vector.tensor_copy(
                            out=kt[j // 4][(j % 4) * 32 : (j % 4) * 32 + NV, cols],
                            in_=zp[0:NV, j :: 32],
                        )

            # ---- top MLP ---------------------------------------------------
            sig = mybir.ActivationFunctionType.Sigmoid
            rk = [xall] + kt
            kps = [128] * (NZTILES + 1)
            th1 = [hp.tile([P, S], FP, tag="hA", name=f"th1_{i}") for i in range(8)]
            nm = 8
            dout = 1024
            for m in range(nm):
                ps = mpp.tile([P, S], FP, tag="mlp")
                for k in range(NZTILES + 1):
                    nc.tensor.matmul(
                        out=ps[:, :],
                        lhsT=tw_t[0][:, k * dout + m * P : k * dout + (m + 1) * P],
                        rhs=rk[k][:, :],
                        start=(k == 0), stop=(k == NZTILES),
                    )
                nc.scalar.activation(out=th1[m][:], in_=ps[:], func=relu,
                                     bias=tb_t[0][0:P, m : m + 1])
            th2 = [hp.tile([P, S], FP, tag="hB", name=f"th2_{i}") for i in range(8)]
            mlp_layer(th1, 1024, tw_t[1], tb_t[1], 1024, relu, th2)
            th3 = [hp.tile([P, S], FP, tag="hA", name=f"th3_{i}") for i in range(4)]
            mlp_layer(th2, 1024, tw_t[2], tb_t[2], 512, relu, th3)
            th4 = [hp.tile([P, S], FP, tag="hB", name=f"th4_{i}") for i in range(2)]
            mlp_layer(th3, 512, tw_t[3], tb_t[3], 256, relu, th4)
            ps5 = mpp.tile([P, S], FP, tag="mlp")
            for k in range(2):
                nc.tensor.matmul(
                    out=ps5[0:1, :], lhsT=tw_t[4][:, k : k + 1], rhs=th4[k][:],
                    start=(k == 0), stop=(k == 1),
                )
            yt = ktp.tile([1, S], FP, tag="y")
            nc.scalar.activation(out=yt[0:1, :], in_=ps5[0:1, :], func=sig,
                                 bias=tb_t[4][0:1, 0:1])
            nc.sync.dma_start(out=y_d[:], in_=yt[0:1, :])

    _split_multi_waits(nc)
    return nc


_NC_CACHE = None


def _get_nc():
    global _NC_CACHE
    if _NC_CACHE is None:
        _NC_CACHE = _build_nc()
    return _NC_CACHE


def _prep_weights(bot_params, top_params):
    """Host-side weight/bias packing. Returns dict of np arrays."""
    out = {}
    for li, (W, b) in enumerate(bot_params):
        W = np.asarray(W, np.float32)
        b = np.asarray(b, np.float32)
        din, dout = W.shape[1], W.shape[0]
        nk = max(1, din // P)
        kp = min(din, P)
        wt = np.zeros((kp, nk * dout), np.float32)
        for k in range(nk):
            wt[: min(P, din - k * P), k * dout : (k + 1) * dout] = W.T[
                k * P : k * P + min(P, din - k * P), :
            ]
        out[f"bw{li}"] = wt
        nm = max(1, dout // P)
        bb = np.zeros((P, nm), np.float32)
        for m in range(nm):
            mw = min(P, dout - m * P)
            bb[:mw, m] = b[m * P : m * P + mw]
        out[f"bb{li}"] = bb

    # tril permutation for top layer 0: reference feature 128+k (k-th tril
    # pair, row-major by i) -> our feature 128 + _tj(j) + (i-j-1)
    li_, lj_ = np.tril_indices(NV, -1)
    for li, (W, b) in enumerate(top_params):
        W = np.asarray(W, np.float32)
        b = np.asarray(b, np.float32)
        if li == 0:
            Wp = np.zeros((W.shape[0], NFP), np.float32)
            Wp[:, :D] = W[:, :D]
            for k in range(NZ):
                i, j = int(li_[k]), int(lj_[k])
                Wp[:, D + (j // 4) * P + (j % 4) * 32 + i] = W[:, D + k]
            W = Wp
        din, dout = W.shape[1], W.shape[0]
        nk = (din + P - 1) // P
        wt = np.zeros((P, nk * dout), np.float32)
        for k in range(nk):
            kp = min(P, din - k * P)
            wt[:kp, k * dout : (k + 1) * dout] = W.T[k * P : k * P + kp, :]
        out[f"tw{li}"] = wt
        nm = max(1, dout // P)
        bb = np.zeros((P, nm), np.float32)
        for m in range(nm):
            mw = min(P, dout - m * P)
            bb[:mw, m] = b[m * P : m * P + mw]
        out[f"tb{li}"] = bb
    return out


def kernel(dense_x, emb_tables, bot_params, top_params,
           sparse_features_indices, sparse_features_offsets):
    dense_x = np.asarray(dense_x, np.float32)
    tbl = np.ascontiguousarray(np.asarray(emb_tables, np.float32).reshape(T * V, D))
    idx = np.asarray(sparse_features_indices, np.int64).reshape(T, B, 4)
    gg = (idx + (np.arange(T, dtype=np.int64) * V)[:, None, None]).astype(np.int32)

    wmap = _prep_weights(bot_params, top_params)
    nc = _get_nc()

    in_maps = []
    for c in range(NCORES):
        arr = gg[:, c * S : (c + 1) * S, :].reshape(T, CHUNKS, P, 4)
        idxp = np.ascontiguousarray(
            arr.transpose(2, 1, 0, 3).reshape(P, CHUNKS * T * 4)
        )
        dxt = np.ascontiguousarray(dense_x[c * S : (c + 1) * S, :].T)
        m = {"tbl": tbl, "idx": idxp, "dxt": dxt}
        m.update(wmap)
        in_maps.append(m)

    kw = {}
    if os.environ.get("KERNEL_TRACE"):
        kw = {"trace": True, "tmpdir": os.environ.get("KERNEL_TRACE_DIR") or None}
    res = run_bass_kernel_spmd(nc, in_maps, list(range(NCORES)), **kw)
    if res.exec_time_ns is not None:
        print(f"HW exec time: {res.exec_time_ns} ns")
    y = np.concatenate([res.results[c]["y"][0] for c in range(NCORES)])
    return y.reshape(B, 1).astype(np.float32)


if __name__ == "__main__":
    pass


# revision 9
# speedup vs baseline: 1.6978x; 1.6978x over previous
"""DLRM forward (embedding_lookup) on 8 Trainium2 NeuronCores.

Strategy: batch-parallel SPMD. Each core owns B/8 = 512 samples and a full
replica of the stacked embedding tables. Per core:
  - bottom MLP (13->512->256->128) in [features, samples] orientation
  - embedding gather via wide indirect DMAs (1 MB each: 4 tables x 1
    sample-chunk x 4 indices x 128 samples), pooling-by-4 via one strided
    DVE tensor_reduce per (table, chunk)
  - PE transpose pooled [samples, D] -> [D, samples] into a per-chunk
    concatenated T matrix [128, 27*128]
  - pairwise interaction as per-sample 27x27 matmuls (strided lhsT AP)
  - lower-triangle extraction via 27 strided PSUM->SBUF copies per half-chunk
  - top MLP (479->1024->1024->512->256->1) with the tril selection absorbed
    into a host-side permutation of the first layer's weight columns
No cross-core collectives are needed.
"""

import os
import sys

import ml_dtypes
import numpy as np

for _p in ("/opt/trn_rl_repo", "/root/.axon_site/_ro/trn_rl_repo"):
    if os.path.isdir(_p) and _p not in sys.path:
        sys.path.append(_p)

import concourse.bass as bass
import concourse.mybir as mybir
import concourse.tile as tile
from concourse.bass_utils import run_bass_kernel_spmd
from concourse.masks import make_identity

T = 26          # embedding tables
V = 100000      # vocab per table
D = 128         # embedding dim
B = 4096        # global batch
NCORES = 8
S = B // NCORES         # samples per core
CHUNKS = 4              # 128-sample chunks per core
P = 128
NV = T + 1              # vectors per sample (x + 26 tables)
NZ = (NV * (NV - 1)) // 2   # 351 strictly-lower interaction features
NF = D + NZ             # 479 top-MLP input features (reference)
NZTILES = 7             # z K-tiles: 4 column-groups per tile at 32-aligned slots
NFP = D + NZTILES * P   # 1024: padded top-MLP K dim on device
FP = mybir.dt.float32
BF = mybir.dt.bfloat16
I32 = mybir.dt.int32

# gather DMA groups per chunk: tables packed 4 per DMA (16 idx/partition = 1 MB)
GROUPS = [(0, 4), (4, 4), (8, 4), (12, 4), (16, 4), (20, 4), (24, 2)]

BOT_DIMS = [13, 512, 256, 128]
TOP_DIMS = [NFP, 1024, 1024, 512, 256, 1]


def _tj(j):
    # feature offset of Z column-group j in our j-major tril ordering
    return 26 * j - (j * (j - 1)) // 2


def _split_multi_waits(nc, max_waits=1):
    """Walrus codegen here only accepts one sync wait per instruction.
    Hoist extra waits onto same-engine NoOps placed just before."""
    for f in nc.m.functions:
        for blk in f.blocks:
            new_insts = []
            for inst in blk.instructions:
                si = inst.sync_info
                if si is not None and si.on_wait and len(si.on_wait) > max_waits:
                    waits = list(si.on_wait)
                    for i, w in enumerate(waits[:-max_waits]):
                        new_insts.append(
                            mybir.InstNoOp(
                                name=f"{inst.name}-ws{i}",
                                engine=inst.engine,
                                sync_info=mybir.SyncInfo(on_wait=[w], on_update=[]),
                                bass_nofuse=True,
                            )
                        )
                    si.on_wait = waits[-max_waits:]
                new_insts.append(inst)
            blk.instructions = new_insts


def _build_nc():
    nc = bass.Bass()
    tbl = nc.declare_dram_parameter("tbl", [T * V, D], FP, isOutput=False)
    idx_d = nc.declare_dram_parameter("idx", [P, CHUNKS * T * 4], I32, isOutput=False)
    dxt_d = nc.declare_dram_parameter("dxt", [BOT_DIMS[0], S], BF, isOutput=False)
    # bottom weights: per layer one tile [128, nk*M]; biases [128, n_mtiles]
    bw_d, bb_d = [], []
    for li in range(3):
        din, dout = BOT_DIMS[li], BOT_DIMS[li + 1]
        nk = max(1, din // P)
        kp = min(din, P)
        bw_d.append(nc.declare_dram_parameter(f"bw{li}", [kp, nk * dout], BF, isOutput=False))
        bb_d.append(nc.declare_dram_parameter(f"bb{li}", [P, max(1, dout // P)], FP, isOutput=False))
    tw_d, tb_d = [], []
    for li in range(5):
        din, dout = TOP_DIMS[li], TOP_DIMS[li + 1]
        nk = (din + P - 1) // P
        tw_d.append(nc.declare_dram_parameter(f"tw{li}", [P, nk * dout], BF, isOutput=False))
        tb_d.append(nc.declare_dram_parameter(f"tb{li}", [P, max(1, dout // P)], FP, isOutput=False))
    y_d = nc.declare_dram_parameter("y", [1, S], FP, isOutput=True)

    with tile.TileContext(nc) as tc:
        with (
            tc.tile_pool(name="wp", bufs=1) as wp,
            tc.tile_pool(name="gp", bufs=3) as gp,
            tc.tile_pool(name="pp", bufs=3) as pp,
            tc.tile_pool(name="ttp", bufs=2) as ttp,
            tc.tile_pool(name="hp", bufs=8) as hp,
            tc.tile_pool(name="ktp", bufs=1) as ktp,
            tc.tile_pool(name="zp", bufs=1, space="PSUM") as zpp,
            tc.tile_pool(name="trp", bufs=2, space="PSUM") as trpp,
            tc.tile_pool(name="mp", bufs=2, space="PSUM") as mpp,
        ):
            # ---- resident loads -------------------------------------------
            idx_t = wp.tile([P, CHUNKS * T * 4], I32, tag="idx")
            nc.sync.dma_start(out=idx_t[:], in_=idx_d[:])
            dxt_t = wp.tile([BOT_DIMS[0], S], BF, tag="dxt")
            nc.sync.dma_start(out=dxt_t[:], in_=dxt_d[:])
            ident = wp.tile([P, P], BF, tag="ident")
            make_identity(nc, ident[:])
            bw_t, bb_t, tw_t, tb_t = [], [], [], []
            for li in range(3):
                w = wp.tile(list(bw_d[li].shape), BF, tag=f"bw{li}", name=f"bwt{li}")
                nc.sync.dma_start(out=w[:], in_=bw_d[li][:])
                bw_t.append(w)
                b = wp.tile(list(bb_d[li].shape), FP, tag=f"bb{li}", name=f"bbt{li}")
                nc.sync.dma_start(out=b[:], in_=bb_d[li][:])
                bb_t.append(b)
            for li in range(5):
                w = wp.tile(list(tw_d[li].shape), BF, tag=f"tw{li}", name=f"twt{li}")
                nc.sync.dma_start(out=w[:], in_=tw_d[li][:])
                tw_t.append(w)
                b = wp.tile(list(tb_d[li].shape), FP, tag=f"tb{li}", name=f"tbt{li}")
                nc.sync.dma_start(out=b[:], in_=tb_d[li][:])
                tb_t.append(b)

            # ---- bottom MLP: dxt [13, S] -> xall [128, S] ------------------
            def mlp_layer(h_in_tiles, kdim, w_tile, b_tile, dout, func, out_tiles, kp_first=None):
                nk = len(h_in_tiles)
                nm = (dout + P - 1) // P
                for m in range(nm):
                    mw = min(P, dout - m * P)
                    ps = mpp.tile([P, S], FP, tag="mlp")
                    for k in range(nk):
                        kp = kp_first if (kp_first is not None and nk == 1) else min(P, kdim - k * P)
                        nc.tensor.matmul(
                            out=ps[0:mw, :],
                            lhsT=w_tile[0:kp, k * dout + m * P : k * dout + m * P + mw],
                            rhs=h_in_tiles[k][0:kp, :],
                            start=(k == 0),
                            stop=(k == nk - 1),
                        )
                    nc.scalar.activation(
                        out=out_tiles[m][0:mw, :],
                        in_=ps[0:mw, :],
                        func=func,
                        bias=b_tile[0:mw, m : m + 1],
                    )

            relu = mybir.ActivationFunctionType.Relu
            h1 = [hp.tile([P, S], BF, tag="hA", name=f"h1_{i}") for i in range(4)]
            mlp_layer([dxt_t], 13, bw_t[0], bb_t[0], 512, relu, h1, kp_first=13)
            h2 = [hp.tile([P, S], BF, tag="hB", name=f"h2_{i}") for i in range(2)]
            mlp_layer(h1, 512, bw_t[1], bb_t[1], 256, relu, h2)
            xall = ktp.tile([P, S], BF, tag="kt0")
            mlp_layer(h2, 256, bw_t[2], bb_t[2], 128, relu, [xall])

            # ---- R feature K-tiles (zeroed: unwritten slot rows must be 0) -
            kt = [ktp.tile([P, S], BF, tag=f"kt{n}", name=f"kt_{n}")
                  for n in range(1, NZTILES + 1)]
            for n in range(NZTILES):
                nc.gpsimd.memset(kt[n][:], 0.0)

            # ---- per chunk: gather -> pool -> transpose -> interact -> extract
            for cc in range(CHUNKS):
                tt = ttp.tile([P, NV * P], BF, tag="tt")
                # x block
                nc.vector.tensor_copy(out=tt[:, 0:P], in_=xall[:, cc * P : (cc + 1) * P])
                for (t0, nt) in GROUPS:
                    g = gp.tile([P, 4 * nt * P], FP, tag="g")
                    c0 = cc * (T * 4) + t0 * 4
                    nc.gpsimd.indirect_dma_start(
                        out=g[:],
                        out_offset=None,
                        in_=tbl[:],
                        in_offset=bass.IndirectOffsetOnAxis(
                            ap=idx_t[:, c0 : c0 + 4 * nt], axis=0
                        ),
                    )
                    for gi in range(nt):
                        t = t0 + gi
                        g0 = gi * 512
                        pa = pp.tile([P, P], FP, tag="pa")
                        nc.any.tensor_add(out=pa[:], in0=g[:, g0 : g0 + P],
                                          in1=g[:, g0 + P : g0 + 2 * P])
                        pb = pp.tile([P, P], FP, tag="pb")
                        nc.any.tensor_add(out=pb[:], in0=g[:, g0 + 2 * P : g0 + 3 * P],
                                          in1=g[:, g0 + 3 * P : g0 + 4 * P])
                        pt = pp.tile([P, P], BF, tag="p")
                        nc.any.tensor_add(out=pt[:], in0=pa[:], in1=pb[:])
                        tr = trpp.tile([P, P], BF, tag="tr")
                        nc.tensor.transpose(out=tr[:], in_=pt[:], identity=ident[:])
                        nc.vector.tensor_copy(
                            out=tt[:, (t + 1) * P : (t + 2) * P], in_=tr[:]
                        )
                # interaction + extraction in two 64-sample halves
                for h in range(2):
                    zp = zpp.tile([P, 2048], FP, tag="z")
                    for sl in range(64):
                        s = h * 64 + sl
                        lhsT = tt[:, s :: P]
                        nc.tensor.matmul(
                            out=zp[0:NV, 32 * sl : 32 * sl + NV],
                            lhsT=lhsT, rhs=lhsT, start=True, stop=True,
                        )
                    cols = slice(cc * P + h * 64, cc * P + h * 64 + 64)
                    # column-group j of Z -> 32-aligned slot (engines require
                    # 32-aligned partition bases); upper-triangle/diagonal
                    # rows are killed by zeroed W1 columns host-side
                    for j in range(NV - 1):
                        nc.vector.tensor_copy(
                            out=kt[j // 4][(j % 4) * 32 : (j % 4) * 32 + NV, cols],
                            in_=zp[0:NV, j :: 32],
                        )

            # ---- top MLP ---------------------------------------------------
            sig = mybir.ActivationFunctionType.Sigmoid
            rk = [xall] + kt
            kps = [128] * (NZTILES + 1)
            th1 = [hp.tile([P, S], BF, tag="hA", name=f"th1_{i}") for i in range(8)]
            nm = 8
            dout = 1024
            for m in range(nm):
                ps = mpp.tile([P, S], FP, tag="mlp")
                for k in range(NZTILES + 1):
                    nc.tensor.matmul(
                        out=ps[:, :],
                        lhsT=tw_t[0][:, k * dout + m * P : k * dout + (m + 1) * P],
                        rhs=rk[k][:, :],
                        start=(k == 0), stop=(k == NZTILES),
                    )
                nc.scalar.activation(out=th1[m][:], in_=ps[:], func=relu,
                                     bias=tb_t[0][0:P, m : m + 1])
            th2 = [hp.tile([P, S], BF, tag="hB", name=f"th2_{i}") for i in range(8)]
            mlp_layer(th1, 1024, tw_t[1], tb_t[1], 1024, relu, th2)
            th3 = [hp.tile([P, S], BF, tag="hA", name=f"th3_{i}") for i in range(4)]
            mlp_layer(th2, 1024, tw_t[2], tb_t[2], 512, relu, th3)
            th4 = [hp.tile([P, S], BF, tag="hB", name=f"th4_{i}") for i in range(2)]
            mlp_layer(th3, 512, tw_t[3], tb_t[3], 256, relu, th4)
            ps5 = mpp.tile([P, S], FP, tag="mlp")
            for k in range(2):
                nc.tensor.matmul(
                    out=ps5[0:1, :], lhsT=tw_t[4][:, k : k + 1], rhs=th4[k][:],
                    start=(k == 0), stop=(k == 1),
                )
            yt = ktp.tile([1, S], FP, tag="y")
            nc.scalar.activation(out=yt[0:1, :], in_=ps5[0:1, :], func=sig,
                                 bias=tb_t[4][0:1, 0:1])
            nc.sync.dma_start(out=y_d[:], in_=yt[0:1, :])

    _split_multi_waits(nc)
    return nc


_NC_CACHE = None


def _get_nc():
    global _NC_CACHE
    if _NC_CACHE is None:
        _NC_CACHE = _build_nc()
    return _NC_CACHE


def _prep_weights(bot_params, top_params):
    """Host-side weight/bias packing. Returns dict of np arrays."""
    out = {}
    for li, (W, b) in enumerate(bot_params):
        W = np.asarray(W, np.float32)
        b = np.asarray(b, np.float32)
        din, dout = W.shape[1], W.shape[0]
        nk = max(1, din // P)
        kp = min(din, P)
        wt = np.zeros((kp, nk * dout), np.float32)
        for k in range(nk):
            wt[: min(P, din - k * P), k * dout : (k + 1) * dout] = W.T[
                k * P : k * P + min(P, din - k * P), :
            ]
        out[f"bw{li}"] = wt.astype(ml_dtypes.bfloat16)
        nm = max(1, dout // P)
        bb = np.zeros((P, nm), np.float32)
        for m in range(nm):
            mw = min(P, dout - m * P)
            bb[:mw, m] = b[m * P : m * P + mw]
        out[f"bb{li}"] = bb

    # tril permutation for top layer 0: reference feature 128+k (k-th tril
    # pair, row-major by i) -> our feature 128 + _tj(j) + (i-j-1)
    li_, lj_ = np.tril_indices(NV, -1)
    for li, (W, b) in enumerate(top_params):
        W = np.asarray(W, np.float32)
        b = np.asarray(b, np.float32)
        if li == 0:
            Wp = np.zeros((W.shape[0], NFP), np.float32)
            Wp[:, :D] = W[:, :D]
            for k in range(NZ):
                i, j = int(li_[k]), int(lj_[k])
                Wp[:, D + (j // 4) * P + (j % 4) * 32 + i] = W[:, D + k]
            W = Wp
        din, dout = W.shape[1], W.shape[0]
        nk = (din + P - 1) // P
        wt = np.zeros((P, nk * dout), np.float32)
        for k in range(nk):
            kp = min(P, din - k * P)
            wt[:kp, k * dout : (k + 1) * dout] = W.T[k * P : k * P + kp, :]
        out[f"tw{li}"] = wt.astype(ml_dtypes.bfloat16)
        nm = max(1, dout // P)
        bb = np.zeros((P, nm), np.float32)
        for m in range(nm):
            mw = min(P, dout - m * P)
            bb[:mw, m] = b[m * P : m * P + mw]
        out[f"tb{li}"] = bb
    return out


def kernel(dense_x, emb_tables, bot_params, top_params,
           sparse_features_indices, sparse_features_offsets):
    dense_x = np.asarray(dense_x, np.float32)
    tbl = np.ascontiguousarray(np.asarray(emb_tables, np.float32).reshape(T * V, D))
    idx = np.asarray(sparse_features_indices, np.int64).reshape(T, B, 4)
    gg = (idx + (np.arange(T, dtype=np.int64) * V)[:, None, None]).astype(np.int32)

    wmap = _prep_weights(bot_params, top_params)
    nc = _get_nc()

    in_maps = []
    for c in range(NCORES):
        arr = gg[:, c * S : (c + 1) * S, :].reshape(T, CHUNKS, P, 4)
        idxp = np.ascontiguousarray(
            arr.transpose(2, 1, 0, 3).reshape(P, CHUNKS * T * 4)
        )
        dxt = np.ascontiguousarray(dense_x[c * S : (c + 1) * S, :].T).astype(ml_dtypes.bfloat16)
        m = {"tbl": tbl, "idx": idxp, "dxt": dxt}
        m.update(wmap)
        in_maps.append(m)

    kw = {}
    if os.environ.get("KERNEL_TRACE"):
        kw = {"trace": True, "tmpdir": os.environ.get("KERNEL_TRACE_DIR") or None}
    res = run_bass_kernel_spmd(nc, in_maps, list(range(NCORES)), **kw)
    if res.exec_time_ns is not None:
        print(f"HW exec time: {res.exec_time_ns} ns")
    y = np.concatenate([res.results[c]["y"][0] for c in range(NCORES)])
    return y.reshape(B, 1).astype(np.float32)


if __name__ == "__main__":
    pass


# revision 11
# speedup vs baseline: 1.8105x; 1.0664x over previous
"""DLRM forward (embedding_lookup) on 8 Trainium2 NeuronCores.

Strategy: batch-parallel SPMD. Each core owns B/8 = 512 samples and a full
replica of the stacked embedding tables. Per core:
  - bottom MLP (13->512->256->128) in [features, samples] orientation
  - embedding gather via wide indirect DMAs (1 MB each: 4 tables x 1
    sample-chunk x 4 indices x 128 samples), pooling-by-4 via one strided
    DVE tensor_reduce per (table, chunk)
  - PE transpose pooled [samples, D] -> [D, samples] into a per-chunk
    concatenated T matrix [128, 27*128]
  - pairwise interaction as per-sample 27x27 matmuls (strided lhsT AP)
  - lower-triangle extraction via 27 strided PSUM->SBUF copies per half-chunk
  - top MLP (479->1024->1024->512->256->1) with the tril selection absorbed
    into a host-side permutation of the first layer's weight columns
No cross-core collectives are needed.
"""

import os
import sys

import ml_dtypes
import numpy as np

for _p in ("/opt/trn_rl_repo", "/root/.axon_site/_ro/trn_rl_repo"):
    if os.path.isdir(_p) and _p not in sys.path:
        sys.path.append(_p)

import concourse.bass as bass
import concourse.mybir as mybir
import concourse.tile as tile
from concourse.bass_utils import run_bass_kernel_spmd
from concourse.masks import make_identity

T = 26          # embedding tables
V = 100000      # vocab per table
D = 128         # embedding dim
B = 4096        # global batch
NCORES = 8
S = B // NCORES         # samples per core
CHUNKS = 4              # 128-sample chunks per core
P = 128
NV = T + 1              # vectors per sample (x + 26 tables)
NZ = (NV * (NV - 1)) // 2   # 351 strictly-lower interaction features
NF = D + NZ             # 479 top-MLP input features (reference)
NZTILES = 7             # z K-tiles: 4 column-groups per tile at 32-aligned slots
NFP = D + NZTILES * P   # 1024: padded top-MLP K dim on device
FP = mybir.dt.float32
BF = mybir.dt.bfloat16
I32 = mybir.dt.int32

# gather DMA groups per chunk: tables packed 4 per DMA (16 idx/partition = 1 MB)
GROUPS = [(0, 4), (4, 4), (8, 4), (12, 4), (16, 4), (20, 4), (24, 2)]

BOT_DIMS = [13, 512, 256, 128]
TOP_DIMS = [NFP, 1024, 1024, 512, 256, 1]


def _tj(j):
    # feature offset of Z column-group j in our j-major tril ordering
    return 26 * j - (j * (j - 1)) // 2


def _split_multi_waits(nc, max_waits=1):
    """Walrus codegen here only accepts one sync wait per instruction.
    Hoist extra waits onto same-engine NoOps placed just before."""
    for f in nc.m.functions:
        for blk in f.blocks:
            new_insts = []
            for inst in blk.instructions:
                si = inst.sync_info
                if si is not None and si.on_wait and len(si.on_wait) > max_waits:
                    waits = list(si.on_wait)
                    for i, w in enumerate(waits[:-max_waits]):
                        new_insts.append(
                            mybir.InstNoOp(
                                name=f"{inst.name}-ws{i}",
                                engine=inst.engine,
                                sync_info=mybir.SyncInfo(on_wait=[w], on_update=[]),
                                bass_nofuse=True,
                            )
                        )
                    si.on_wait = waits[-max_waits:]
                new_insts.append(inst)
            blk.instructions = new_insts


def _build_nc():
    nc = bass.Bass()
    tbl = nc.declare_dram_parameter("tbl", [T * V, D], FP, isOutput=False)
    idx_d = nc.declare_dram_parameter("idx", [P, CHUNKS * T * 4], I32, isOutput=False)
    dxt_d = nc.declare_dram_parameter("dxt", [BOT_DIMS[0], S], BF, isOutput=False)
    # bottom weights: per layer one tile [128, nk*M]; biases [128, n_mtiles]
    bw_d, bb_d = [], []
    for li in range(3):
        din, dout = BOT_DIMS[li], BOT_DIMS[li + 1]
        nk = max(1, din // P)
        kp = min(din, P)
        bw_d.append(nc.declare_dram_parameter(f"bw{li}", [kp, nk * dout], BF, isOutput=False))
        bb_d.append(nc.declare_dram_parameter(f"bb{li}", [P, max(1, dout // P)], FP, isOutput=False))
    tw_d, tb_d = [], []
    for li in range(5):
        din, dout = TOP_DIMS[li], TOP_DIMS[li + 1]
        nk = (din + P - 1) // P
        tw_d.append(nc.declare_dram_parameter(f"tw{li}", [P, nk * dout], BF, isOutput=False))
        tb_d.append(nc.declare_dram_parameter(f"tb{li}", [P, max(1, dout // P)], FP, isOutput=False))
    y_d = nc.declare_dram_parameter("y", [1, S], FP, isOutput=True)

    with tile.TileContext(nc) as tc:
        with (
            tc.tile_pool(name="wp", bufs=1) as wp,
            tc.tile_pool(name="gp", bufs=4) as gp,
            tc.tile_pool(name="pp", bufs=3) as pp,
            tc.tile_pool(name="ttp", bufs=2) as ttp,
            tc.tile_pool(name="hp", bufs=8) as hp,
            tc.tile_pool(name="ktp", bufs=1) as ktp,
            tc.tile_pool(name="zp", bufs=1, space="PSUM") as zpp,
            tc.tile_pool(name="trp", bufs=2, space="PSUM") as trpp,
            tc.tile_pool(name="mp", bufs=2, space="PSUM") as mpp,
        ):
            # ---- resident loads -------------------------------------------
            idx_t = wp.tile([P, CHUNKS * T * 4], I32, tag="idx")
            nc.sync.dma_start(out=idx_t[:], in_=idx_d[:])
            dxt_t = wp.tile([BOT_DIMS[0], S], BF, tag="dxt")
            nc.sync.dma_start(out=dxt_t[:], in_=dxt_d[:])
            ident = wp.tile([P, P], BF, tag="ident")
            make_identity(nc, ident[:])
            bw_t, bb_t, tw_t, tb_t = [], [], [], []
            for li in range(3):
                w = wp.tile(list(bw_d[li].shape), BF, tag=f"bw{li}", name=f"bwt{li}")
                nc.sync.dma_start(out=w[:], in_=bw_d[li][:])
                bw_t.append(w)
                b = wp.tile(list(bb_d[li].shape), FP, tag=f"bb{li}", name=f"bbt{li}")
                nc.sync.dma_start(out=b[:], in_=bb_d[li][:])
                bb_t.append(b)
            for li in range(5):
                w = wp.tile(list(tw_d[li].shape), BF, tag=f"tw{li}", name=f"twt{li}")
                nc.sync.dma_start(out=w[:], in_=tw_d[li][:])
                tw_t.append(w)
                b = wp.tile(list(tb_d[li].shape), FP, tag=f"tb{li}", name=f"tbt{li}")
                nc.sync.dma_start(out=b[:], in_=tb_d[li][:])
                tb_t.append(b)

            # ---- bottom MLP: dxt [13, S] -> xall [128, S] ------------------
            def mlp_layer(h_in_tiles, kdim, w_tile, b_tile, dout, func, out_tiles, kp_first=None):
                nk = len(h_in_tiles)
                nm = (dout + P - 1) // P
                for m in range(nm):
                    mw = min(P, dout - m * P)
                    ps = mpp.tile([P, S], FP, tag="mlp")
                    for k in range(nk):
                        kp = kp_first if (kp_first is not None and nk == 1) else min(P, kdim - k * P)
                        nc.tensor.matmul(
                            out=ps[0:mw, :],
                            lhsT=w_tile[0:kp, k * dout + m * P : k * dout + m * P + mw],
                            rhs=h_in_tiles[k][0:kp, :],
                            start=(k == 0),
                            stop=(k == nk - 1),
                        )
                    nc.scalar.activation(
                        out=out_tiles[m][0:mw, :],
                        in_=ps[0:mw, :],
                        func=func,
                        bias=b_tile[0:mw, m : m + 1],
                    )

            relu = mybir.ActivationFunctionType.Relu
            h1 = [hp.tile([P, S], BF, tag="hA", name=f"h1_{i}") for i in range(4)]
            mlp_layer([dxt_t], 13, bw_t[0], bb_t[0], 512, relu, h1, kp_first=13)
            h2 = [hp.tile([P, S], BF, tag="hB", name=f"h2_{i}") for i in range(2)]
            mlp_layer(h1, 512, bw_t[1], bb_t[1], 256, relu, h2)
            xall = ktp.tile([P, S], BF, tag="kt0")
            mlp_layer(h2, 256, bw_t[2], bb_t[2], 128, relu, [xall])

            # ---- R feature K-tiles (zeroed: unwritten slot rows must be 0) -
            kt = [ktp.tile([P, S], BF, tag=f"kt{n}", name=f"kt_{n}")
                  for n in range(1, NZTILES + 1)]
            for n in range(NZTILES):
                nc.gpsimd.memset(kt[n][:], 0.0)

            # ---- per chunk: gather -> pool -> transpose -> interact -> extract
            for cc in range(CHUNKS):
                tt = ttp.tile([P, NV * P], BF, tag="tt")
                # x block
                nc.vector.tensor_copy(out=tt[:, 0:P], in_=xall[:, cc * P : (cc + 1) * P])
                for (t0, nt) in GROUPS:
                    g = gp.tile([P, 4 * nt * P], FP, tag="g")
                    c0 = cc * (T * 4) + t0 * 4
                    nc.gpsimd.indirect_dma_start(
                        out=g[:],
                        out_offset=None,
                        in_=tbl[:],
                        in_offset=bass.IndirectOffsetOnAxis(
                            ap=idx_t[:, c0 : c0 + 4 * nt], axis=0
                        ),
                    )
                    for gi in range(nt):
                        t = t0 + gi
                        g0 = gi * 512
                        pa = pp.tile([P, P], FP, tag="pa")
                        nc.any.tensor_add(out=pa[:], in0=g[:, g0 : g0 + P],
                                          in1=g[:, g0 + P : g0 + 2 * P])
                        pb = pp.tile([P, P], FP, tag="pb")
                        nc.any.tensor_add(out=pb[:], in0=g[:, g0 + 2 * P : g0 + 3 * P],
                                          in1=g[:, g0 + 3 * P : g0 + 4 * P])
                        pt = pp.tile([P, P], BF, tag="p")
                        nc.any.tensor_add(out=pt[:], in0=pa[:], in1=pb[:])
                        tr = trpp.tile([P, P], BF, tag="tr")
                        nc.tensor.transpose(out=tr[:], in_=pt[:], identity=ident[:])
                        nc.scalar.activation(
                            out=tt[:, (t + 1) * P : (t + 2) * P], in_=tr[:],
                            func=mybir.ActivationFunctionType.Copy,
                        )
                # interaction + extraction in two 64-sample halves
                for h in range(2):
                    zp = zpp.tile([P, 2048], FP, tag="z")
                    for sl in range(64):
                        s = h * 64 + sl
                        lhsT = tt[:, s :: P]
                        nc.tensor.matmul(
                            out=zp[0:NV, 32 * sl : 32 * sl + NV],
                            lhsT=lhsT, rhs=lhsT, start=True, stop=True,
                        )
                    cols = slice(cc * P + h * 64, cc * P + h * 64 + 64)
                    # column-group j of Z -> 32-aligned slot (engines require
                    # 32-aligned partition bases); upper-triangle/diagonal
                    # rows are killed by zeroed W1 columns host-side
                    for j in range(NV - 1):
                        nc.vector.tensor_copy(
                            out=kt[j // 4][(j % 4) * 32 : (j % 4) * 32 + NV, cols],
                            in_=zp[0:NV, j :: 32],
                        )

            # ---- top MLP ---------------------------------------------------
            sig = mybir.ActivationFunctionType.Sigmoid
            rk = [xall] + kt
            kps = [128] * (NZTILES + 1)
            th1 = [hp.tile([P, S], BF, tag="hA", name=f"th1_{i}") for i in range(8)]
            nm = 8
            dout = 1024
            for m in range(nm):
                ps = mpp.tile([P, S], FP, tag="mlp")
                for k in range(NZTILES + 1):
                    nc.tensor.matmul(
                        out=ps[:, :],
                        lhsT=tw_t[0][:, k * dout + m * P : k * dout + (m + 1) * P],
                        rhs=rk[k][:, :],
                        start=(k == 0), stop=(k == NZTILES),
                    )
                nc.scalar.activation(out=th1[m][:], in_=ps[:], func=relu,
                                     bias=tb_t[0][0:P, m : m + 1])
            th2 = [hp.tile([P, S], BF, tag="hB", name=f"th2_{i}") for i in range(8)]
            mlp_layer(th1, 1024, tw_t[1], tb_t[1], 1024, relu, th2)
            th3 = [hp.tile([P, S], BF, tag="hA", name=f"th3_{i}") for i in range(4)]
            mlp_layer(th2, 1024, tw_t[2], tb_t[2], 512, relu, th3)
            th4 = [hp.tile([P, S], BF, tag="hB", name=f"th4_{i}") for i in range(2)]
            mlp_layer(th3, 512, tw_t[3], tb_t[3], 256, relu, th4)
            ps5 = mpp.tile([P, S], FP, tag="mlp")
            for k in range(2):
                nc.tensor.matmul(
                    out=ps5[0:1, :], lhsT=tw_t[4][:, k : k + 1], rhs=th4[k][:],
                    start=(k == 0), stop=(k == 1),
                )
            yt = ktp.tile([1, S], FP, tag="y")
            nc.scalar.activation(out=yt[0:1, :], in_=ps5[0:1, :], func=sig,
                                 bias=tb_t[4][0:1, 0:1])
            nc.sync.dma_start(out=y_d[:], in_=yt[0:1, :])

    _split_multi_waits(nc)
    return nc


_NC_CACHE = None


def _get_nc():
    global _NC_CACHE
    if _NC_CACHE is None:
        _NC_CACHE = _build_nc()
    return _NC_CACHE


def _prep_weights(bot_params, top_params):
    """Host-side weight/bias packing. Returns dict of np arrays."""
    out = {}
    for li, (W, b) in enumerate(bot_params):
        W = np.asarray(W, np.float32)
        b = np.asarray(b, np.float32)
        din, dout = W.shape[1], W.shape[0]
        nk = max(1, din // P)
        kp = min(din, P)
        wt = np.zeros((kp, nk * dout), np.float32)
        for k in range(nk):
            wt[: min(P, din - k * P), k * dout : (k + 1) * dout] = W.T[
                k * P : k * P + min(P, din - k * P), :
            ]
        out[f"bw{li}"] = wt.astype(ml_dtypes.bfloat16)
        nm = max(1, dout // P)
        bb = np.zeros((P, nm), np.float32)
        for m in range(nm):
            mw = min(P, dout - m * P)
            bb[:mw, m] = b[m * P : m * P + mw]
        out[f"bb{li}"] = bb

    # tril permutation for top layer 0: reference feature 128+k (k-th tril
    # pair, row-major by i) -> our feature 128 + _tj(j) + (i-j-1)
    li_, lj_ = np.tril_indices(NV, -1)
    for li, (W, b) in enumerate(top_params):
        W = np.asarray(W, np.float32)
        b = np.asarray(b, np.float32)
        if li == 0:
            Wp = np.zeros((W.shape[0], NFP), np.float32)
            Wp[:, :D] = W[:, :D]
            for k in range(NZ):
                i, j = int(li_[k]), int(lj_[k])
                Wp[:, D + (j // 4) * P + (j % 4) * 32 + i] = W[:, D + k]
            W = Wp
        din, dout = W.shape[1], W.shape[0]
        nk = (din + P - 1) // P
        wt = np.zeros((P, nk * dout), np.float32)
        for k in range(nk):
            kp = min(P, din - k * P)
            wt[:kp, k * dout : (k + 1) * dout] = W.T[k * P : k * P + kp, :]
        out[f"tw{li}"] = wt.astype(ml_dtypes.bfloat16)
        nm = max(1, dout // P)
        bb = np.zeros((P, nm), np.float32)
        for m in range(nm):
            mw = min(P, dout - m * P)
            bb[:mw, m] = b[m * P : m * P + mw]
        out[f"tb{li}"] = bb
    return out


def kernel(dense_x, emb_tables, bot_params, top_params,
           sparse_features_indices, sparse_features_offsets):
    dense_x = np.asarray(dense_x, np.float32)
    tbl = np.ascontiguousarray(np.asarray(emb_tables, np.float32).reshape(T * V, D))
    idx = np.asarray(sparse_features_indices, np.int64).reshape(T, B, 4)
    gg = (idx + (np.arange(T, dtype=np.int64) * V)[:, None, None]).astype(np.int32)

    wmap = _prep_weights(bot_params, top_params)
    nc = _get_nc()

    in_maps = []
    for c in range(NCORES):
        arr = gg[:, c * S : (c + 1) * S, :].reshape(T, CHUNKS, P, 4)
        idxp = np.ascontiguousarray(
            arr.transpose(2, 1, 0, 3).reshape(P, CHUNKS * T * 4)
        )
        dxt = np.ascontiguousarray(dense_x[c * S : (c + 1) * S, :].T).astype(ml_dtypes.bfloat16)
        m = {"tbl": tbl, "idx": idxp, "dxt": dxt}
        m.update(wmap)
        in_maps.append(m)

    kw = {}
    if os.environ.get("KERNEL_TRACE"):
        try:  # profiling needs the antenv.axon_hooks shim; never break a plain run
            import antenv.axon_hooks  # noqa: F401
            kw = {"trace": True, "tmpdir": os.environ.get("KERNEL_TRACE_DIR") or None}
        except ImportError:
            pass
    res = run_bass_kernel_spmd(nc, in_maps, list(range(NCORES)), **kw)
    if res.exec_time_ns is not None:
        print(f"HW exec time: {res.exec_time_ns} ns")
    y = np.concatenate([res.results[c]["y"][0] for c in range(NCORES)])
    return y.reshape(B, 1).astype(np.float32)


if __name__ == "__main__":
    pass
